# revision 13
# baseline (speedup 1.0000x reference)
"""Trainium2 Bass kernel for nn_DenseTensor (dense_mlp, bilinear form).

Computes out = x @ W + einsum('bd,due,be->bu', x, V, x) + b with
B=1024, D=U=E=512 on 8 NeuronCores.

Sharding: tensor-parallel over the units axis U — core c owns units
[c*64, (c+1)*64). Each core receives the full x (replicated, as a bf16
x^T for the matmul stationary operand plus an f32 x for the reduce
stage) and its V/W/b shard. No collectives; the host concatenates the
8 disjoint output column-slices.

Per-core dataflow, per unit u:
  PE : A_u = x @ V[:,u,:]  as 8 batch-chunks x 4 K-chunk accumulating
       bf16 matmuls ([128k,128m] @ [128k,512n] -> PSUM f32).
  DVE: one fused tensor_tensor_reduce per batch-chunk:
       quad[b,u] = sum_e A_u[b,e] * x[b,e]  (product + row-reduce in a
       single pass, accumulated straight into the output column).
The linear term x @ W_shard + b is computed once up front (PE + DVE)
and added into the output tile with a single tensor_add at the end.
"""

import sys
import types

import numpy as np
import ml_dtypes

B, D, U = 1024, 512, 512
N_CORES = 8
UPC = U // N_CORES       # units per core = 64
P = 128                  # partitions
BC = B // P              # batch chunks = 8
KC = D // P              # contraction chunks = 4

BF16 = ml_dtypes.bfloat16


def _ensure_axon_hooks():
    """Provide the antenv.axon_hooks registry if the image lacks it.

    concourse.bass_utils imports it unconditionally when tracing is
    requested (e.g. BASS_TRACE=1); without this shim that import path
    raises ModuleNotFoundError.
    """
    try:
        import antenv.axon_hooks  # noqa: F401
        return
    except ImportError:
        pass
    mod = types.ModuleType("antenv.axon_hooks")
    mod._hook = None

    def set_axon_ntff_profile_hook(h):
        mod._hook = h

    def get_axon_ntff_profile_hook():
        return mod._hook

    mod.set_axon_ntff_profile_hook = set_axon_ntff_profile_hook
    mod.get_axon_ntff_profile_hook = get_axon_ntff_profile_hook
    sys.modules["antenv.axon_hooks"] = mod
    try:
        import antenv
        antenv.axon_hooks = mod
    except ImportError:
        pass
    try:
        from trn_agent_boot.trn_boot import _ntff_profile_via_ctypes
        hook = _ntff_profile_via_ctypes("/opt/axon/libaxon_pjrt.so")
        if hook is not None:
            set_axon_ntff_profile_hook(hook)
    except Exception:
        pass


def _split_multi_waits(nc, mybir, max_waits=1):
    """Legalize for walrus builds that allow only one sync wait per
    instruction: move extra waits onto same-engine NoOps placed just
    before the offending instruction (queues are in-order, so this is
    semantics-preserving)."""
    for f in nc.m.functions:
        for blk in f.blocks:
            new_insts, changed = [], False
            for inst in blk.instructions:
                si = inst.sync_info
                if si is not None and len(si.on_wait) > max_waits:
                    waits = list(si.on_wait)
                    extra, keep = waits[:-max_waits], waits[-max_waits:]
                    for j, w in enumerate(extra):
                        new_insts.append(mybir.InstNoOp(
                            name=f"{inst.name}-sw{j}",
                            engine=inst.engine,
                            bass_nofuse=True,
                            sync_info=mybir.SyncInfo(on_wait=[w], on_update=[]),
                        ))
                    inst.sync_info = mybir.SyncInfo(
                        on_wait=keep, on_update=list(si.on_update))
                    changed = True
                new_insts.append(inst)
            if changed:
                blk.instructions = new_insts


def _build_program():
    import concourse.bass as bass
    import concourse.mybir as mybir
    import concourse.tile as tile

    f32 = mybir.dt.float32
    bf16 = mybir.dt.bfloat16

    nc = bass.Bass(trn_type="TRN2")
    xT = nc.dram_tensor("xT", [D, B], bf16, kind="ExternalInput")
    x32 = nc.dram_tensor("x32", [B, D], f32, kind="ExternalInput")
    Vs = nc.dram_tensor("Vs", [UPC, D, D], bf16, kind="ExternalInput")
    Ws = nc.dram_tensor("Ws", [D, UPC], bf16, kind="ExternalInput")
    bs = nc.dram_tensor("bs", [P, UPC], f32, kind="ExternalInput")
    outs = nc.dram_tensor("outs", [B, UPC], f32, kind="ExternalOutput")

    mult = mybir.AluOpType.mult
    add = mybir.AluOpType.add

    with tile.TileContext(nc) as tc:
        with tc.tile_pool(name="const", bufs=1) as cpool:
            xT_sb = cpool.tile([P, KC, B], bf16)
            x32_sb = cpool.tile([P, BC, D], f32)
            ws_sb = cpool.tile([P, KC, UPC], bf16)
            bias_sb = cpool.tile([P, UPC], f32)
            lin_sb = cpool.tile([P, BC, UPC], f32)
            out_sb = cpool.tile([P, BC, UPC], f32)

            # DMA plan: the first two unit-pairs' V tiles go FIRST on the
            # sync queue (they gate the first quad matmuls), then the xT
            # chunks; x32/Ws/bias ride the gpsimd queue in parallel.
            xT_r = xT.rearrange("(k p) b -> p k b", p=P)

            # Units processed in groups of G: the G per-unit matmul
            # accumulations land in one [P, G, D] PSUM tile (G banks), so
            # stage 2 is a single broadcast tensor_mul plus a single
            # batched tensor_reduce per (group, chunk) — DVE op cost is
            # dominated by fixed overhead, so batching banks is ~Gx
            # cheaper than per-unit ops.
            # Units in pairs (G=2): each pair's matmuls land in one
            # [P, 2, D] PSUM tile (2 banks), 4 tiles in flight so the PE
            # never waits on the stage-2 consumer latency. Stage 2 per
            # pair: one broadcast product over both banks (DVE, fixed
            # overhead amortized), then row-reduces — mostly on the
            # Scalar engine (accumulate-activation), every 5th pair as a
            # batched reduce on DVE — keeping both engines well under the
            # PE's cadence.
            G = 2
            Copy = mybir.ActivationFunctionType.Copy
            with tc.tile_pool(name="vp", bufs=3 * G) as vpool, \
                 tc.tile_pool(name="qp", bufs=4, space="PSUM") as qpool, \
                 tc.tile_pool(name="dp", bufs=1) as dpool:
                act_dummy = dpool.tile([P, D], bf16)

                def v_load(u):
                    vt = vpool.tile([P, KC, D], bf16, tag="vt")
                    nc.sync.dma_start(
                        out=vt, in_=Vs[u].rearrange("(k p) e -> p k e", p=P))
                    return vt

                PRE = 2
                pre_vts = {ug: [v_load(ug * G + j) for j in range(G)]
                           for ug in range(PRE)}
                for k in range(KC):
                    nc.sync.dma_start(out=xT_sb[:, k, :], in_=xT_r[:, k, :])
                nc.gpsimd.dma_start(
                    out=ws_sb, in_=Ws.rearrange("(k p) u -> p k u", p=P))
                nc.gpsimd.dma_start(
                    out=x32_sb, in_=x32.rearrange("(c p) d -> p c d", p=P))
                nc.gpsimd.dma_start(out=bias_sb, in_=bs[:, :])

                gidx = 0
                for ug in range(UPC // G):
                    vts = pre_vts.pop(ug) if ug in pre_vts else \
                        [v_load(ug * G + j) for j in range(G)]
                    for bc in range(BC):
                        qg = qpool.tile([P, G, D], f32)
                        for k in range(KC):
                            for j in range(G):
                                nc.tensor.matmul(
                                    qg[:, j, :],
                                    xT_sb[:, k, bc * P:(bc + 1) * P],
                                    vts[j][:, k, :],
                                    start=(k == 0),
                                    stop=(k == KC - 1),
                                )
                        xb = x32_sb[:, bc, :][:, None, :].broadcast_to((P, G, D))
                        nc.vector.tensor_mul(qg, qg, xb)
                        u0 = ug * G
                        if gidx % 5 == 0:
                            nc.vector.tensor_reduce(
                                out_sb[:, bc, u0:u0 + G], qg,
                                mybir.AxisListType.X, add)
                        else:
                            for j in range(G):
                                nc.scalar.activation(
                                    act_dummy, qg[:, j, :], Copy,
                                    accum_out=out_sb[:, bc, u0 + j:u0 + j + 1])
                        gidx += 1

                # Linear term last: its small matmuls run on the PE while
                # the final quad groups drain on DVE/ACT (PE is otherwise
                # idle there). Reuses the quad PSUM slots (same tag) to
                # stay within the 8-bank budget.
                for bc in range(BC):
                    lp = qpool.tile([P, G, D], f32, tag="qg")
                    for k in range(KC):
                        nc.tensor.matmul(
                            lp[:, 0, 0:UPC],
                            xT_sb[:, k, bc * P:(bc + 1) * P],
                            ws_sb[:, k, :],
                            start=(k == 0),
                            stop=(k == KC - 1),
                        )
                    nc.vector.tensor_add(
                        lin_sb[:, bc, :], lp[:, 0, 0:UPC], bias_sb)

            nc.vector.tensor_add(out_sb, out_sb, lin_sb)
            nc.sync.dma_start(
                out=outs.rearrange("(c p) u -> p c u", p=P), in_=out_sb)

    _split_multi_waits(nc, mybir, max_waits=1)
    return nc


_LAST_RUN = {}


def kernel(x, W, V, b):
    _ensure_axon_hooks()
    import concourse.bass_utils as bass_utils
    bass_utils.upload_artifacts = lambda d: f"local:{d}"

    x = np.asarray(x, dtype=np.float32)
    W = np.asarray(W, dtype=np.float32)
    V = np.asarray(V, dtype=np.float32)
    b = np.asarray(b, dtype=np.float32)

    xT_bf = np.ascontiguousarray(x.T).astype(BF16)
    Vt_bf = V.transpose(1, 0, 2).astype(BF16)   # (U, D, E) contiguous bf16
    W_bf = W.astype(BF16)

    in_maps = []
    for c in range(N_CORES):
        us, ue = c * UPC, (c + 1) * UPC
        in_maps.append({
            "xT": xT_bf,
            "x32": x,
            "Vs": np.ascontiguousarray(Vt_bf[us:ue]),
            "Ws": np.ascontiguousarray(W_bf[:, us:ue]),
            "bs": np.ascontiguousarray(
                np.broadcast_to(b[us:ue], (P, UPC))).astype(np.float32),
        })

    nc = _build_program()
    res = None
    last_exc = None
    for attempt in range(3):
        try:
            res = bass_utils.run_bass_kernel_spmd(
                nc, in_maps, core_ids=list(range(N_CORES)))
            break
        except Exception as e:  # transient NRT device errors have been seen
            last_exc = e
    if res is None:
        raise last_exc
    _LAST_RUN["result"] = res

    out = np.concatenate(
        [res.results[c]["outs"] for c in range(N_CORES)], axis=1)
    return out.astype(np.float32)


# revision 17
# speedup vs baseline: 1.0013x; 1.0013x over previous
"""Trainium2 Bass kernel for nn_DenseTensor (dense_mlp, bilinear form).

Computes out = x @ W + einsum('bd,due,be->bu', x, V, x) + b with
B=1024, D=U=E=512 on 8 NeuronCores.

Sharding: tensor-parallel over the units axis U — core c owns units
[c*64, (c+1)*64). Each core receives the full x (replicated, as a bf16
x^T for the matmul stationary operand plus an f32 x for the reduce
stage) and its V/W/b shard. No collectives; the host concatenates the
8 disjoint output column-slices.

Per-core dataflow, per unit u:
  PE : A_u = x @ V[:,u,:]  as 8 batch-chunks x 4 K-chunk accumulating
       bf16 matmuls ([128k,128m] @ [128k,512n] -> PSUM f32).
  DVE: one fused tensor_tensor_reduce per batch-chunk:
       quad[b,u] = sum_e A_u[b,e] * x[b,e]  (product + row-reduce in a
       single pass, accumulated straight into the output column).
The linear term x @ W_shard + b is computed once up front (PE + DVE)
and added into the output tile with a single tensor_add at the end.
"""

import sys
import types

import numpy as np
import ml_dtypes

B, D, U = 1024, 512, 512
N_CORES = 8
UPC = U // N_CORES       # units per core = 64
P = 128                  # partitions
BC = B // P              # batch chunks = 8
KC = D // P              # contraction chunks = 4

BF16 = ml_dtypes.bfloat16


def _ensure_axon_hooks():
    """Provide the antenv.axon_hooks registry if the image lacks it.

    concourse.bass_utils imports it unconditionally when tracing is
    requested (e.g. BASS_TRACE=1); without this shim that import path
    raises ModuleNotFoundError.
    """
    try:
        import antenv.axon_hooks  # noqa: F401
        return
    except ImportError:
        pass
    mod = types.ModuleType("antenv.axon_hooks")
    mod._hook = None

    def set_axon_ntff_profile_hook(h):
        mod._hook = h

    def get_axon_ntff_profile_hook():
        return mod._hook

    mod.set_axon_ntff_profile_hook = set_axon_ntff_profile_hook
    mod.get_axon_ntff_profile_hook = get_axon_ntff_profile_hook
    sys.modules["antenv.axon_hooks"] = mod
    try:
        import antenv
        antenv.axon_hooks = mod
    except ImportError:
        pass
    try:
        from trn_agent_boot.trn_boot import _ntff_profile_via_ctypes
        hook = _ntff_profile_via_ctypes("/opt/axon/libaxon_pjrt.so")
        if hook is not None:
            set_axon_ntff_profile_hook(hook)
    except Exception:
        pass


def _split_multi_waits(nc, mybir, max_waits=1):
    """Legalize for walrus builds that allow only one sync wait per
    instruction: move extra waits onto same-engine NoOps placed just
    before the offending instruction (queues are in-order, so this is
    semantics-preserving)."""
    for f in nc.m.functions:
        for blk in f.blocks:
            new_insts, changed = [], False
            for inst in blk.instructions:
                si = inst.sync_info
                if si is not None and len(si.on_wait) > max_waits:
                    waits = list(si.on_wait)
                    extra, keep = waits[:-max_waits], waits[-max_waits:]
                    for j, w in enumerate(extra):
                        new_insts.append(mybir.InstNoOp(
                            name=f"{inst.name}-sw{j}",
                            engine=inst.engine,
                            bass_nofuse=True,
                            sync_info=mybir.SyncInfo(on_wait=[w], on_update=[]),
                        ))
                    inst.sync_info = mybir.SyncInfo(
                        on_wait=keep, on_update=list(si.on_update))
                    changed = True
                new_insts.append(inst)
            if changed:
                blk.instructions = new_insts


def _build_program():
    import concourse.bass as bass
    import concourse.mybir as mybir
    import concourse.tile as tile

    f32 = mybir.dt.float32
    bf16 = mybir.dt.bfloat16

    nc = bass.Bass(trn_type="TRN2")
    xT = nc.dram_tensor("xT", [D, B], bf16, kind="ExternalInput")
    x32 = nc.dram_tensor("x32", [B, D], f32, kind="ExternalInput")
    Vs = nc.dram_tensor("Vs", [UPC, P, KC, D], bf16, kind="ExternalInput")
    Ws = nc.dram_tensor("Ws", [D, UPC], bf16, kind="ExternalInput")
    bs = nc.dram_tensor("bs", [P, UPC], f32, kind="ExternalInput")
    outs = nc.dram_tensor("outs", [B, UPC], f32, kind="ExternalOutput")

    mult = mybir.AluOpType.mult
    add = mybir.AluOpType.add

    with tile.TileContext(nc) as tc:
        with tc.tile_pool(name="const", bufs=1) as cpool:
            xT_sb = cpool.tile([P, KC, B], bf16)
            x32_sb = cpool.tile([P, BC, D], f32)
            ws_sb = cpool.tile([P, KC, UPC], bf16)
            bias_sb = cpool.tile([P, UPC], f32)
            lin_sb = cpool.tile([P, BC, UPC], f32)
            out_sb = cpool.tile([P, BC, UPC], f32)

            # DMA plan: the first two unit-pairs' V tiles go FIRST on the
            # sync queue (they gate the first quad matmuls), then the xT
            # chunks; x32/Ws/bias ride the gpsimd queue in parallel.
            xT_r = xT.rearrange("(k p) b -> p k b", p=P)

            # Units in pairs (G=2): each pair's matmuls land in one
            # [P, 2, D] PSUM tile (2 banks), 4 tiles in flight so the PE
            # never waits on the stage-2 consumer latency. Stage 2 per
            # pair: one broadcast product over both banks (DVE, fixed
            # overhead amortized), then row-reduces — mostly on the
            # Scalar engine (accumulate-activation), every 5th pair as a
            # batched reduce on DVE — keeping both engines well under the
            # PE's cadence.
            G = 2
            Copy = mybir.ActivationFunctionType.Copy
            with tc.tile_pool(name="vp", bufs=3 * G) as vpool, \
                 tc.tile_pool(name="qp", bufs=4, space="PSUM") as qpool, \
                 tc.tile_pool(name="dp", bufs=1) as dpool:
                act_dummy = dpool.tile([P, D], bf16)

                def v_load(u):
                    # Host pre-permuted V to [p, k, e]: one contiguous 4KB
                    # line per partition per unit -> fast uniform DMAs.
                    vt = vpool.tile([P, KC, D], bf16, tag="vt")
                    nc.sync.dma_start(out=vt, in_=Vs[u])
                    return vt

                PRE = 2
                pre_vts = {ug: [v_load(ug * G + j) for j in range(G)]
                           for ug in range(PRE)}
                for k in range(KC):
                    nc.sync.dma_start(out=xT_sb[:, k, :], in_=xT_r[:, k, :])
                nc.gpsimd.dma_start(
                    out=ws_sb, in_=Ws.rearrange("(k p) u -> p k u", p=P))
                nc.gpsimd.dma_start(
                    out=x32_sb, in_=x32.rearrange("(c p) d -> p c d", p=P))
                nc.gpsimd.dma_start(out=bias_sb, in_=bs[:, :])

                gidx = 0
                for ug in range(UPC // G):
                    vts = pre_vts.pop(ug) if ug in pre_vts else \
                        [v_load(ug * G + j) for j in range(G)]
                    for bc in range(BC):
                        qg = qpool.tile([P, G, D], f32)
                        for k in range(KC):
                            for j in range(G):
                                nc.tensor.matmul(
                                    qg[:, j, :],
                                    xT_sb[:, k, bc * P:(bc + 1) * P],
                                    vts[j][:, k, :],
                                    start=(k == 0),
                                    stop=(k == KC - 1),
                                )
                        xb = x32_sb[:, bc, :][:, None, :].broadcast_to((P, G, D))
                        nc.vector.tensor_mul(qg, qg, xb)
                        u0 = ug * G
                        if gidx % 5 == 0:
                            nc.vector.tensor_reduce(
                                out_sb[:, bc, u0:u0 + G], qg,
                                mybir.AxisListType.X, add)
                        else:
                            for j in range(G):
                                nc.scalar.activation(
                                    act_dummy, qg[:, j, :], Copy,
                                    accum_out=out_sb[:, bc, u0 + j:u0 + j + 1])
                        gidx += 1

                # Linear term last: its small matmuls run on the PE while
                # the final quad groups drain on DVE/ACT (PE is otherwise
                # idle there). Reuses the quad PSUM slots (same tag) to
                # stay within the 8-bank budget.
                for bc in range(BC):
                    lp = qpool.tile([P, G, D], f32, tag="qg")
                    for k in range(KC):
                        nc.tensor.matmul(
                            lp[:, 0, 0:UPC],
                            xT_sb[:, k, bc * P:(bc + 1) * P],
                            ws_sb[:, k, :],
                            start=(k == 0),
                            stop=(k == KC - 1),
                        )
                    nc.vector.tensor_add(
                        lin_sb[:, bc, :], lp[:, 0, 0:UPC], bias_sb)

            nc.vector.tensor_add(out_sb, out_sb, lin_sb)
            nc.sync.dma_start(
                out=outs.rearrange("(c p) u -> p c u", p=P), in_=out_sb)

    _split_multi_waits(nc, mybir, max_waits=1)
    return nc


_LAST_RUN = {}


def kernel(x, W, V, b):
    _ensure_axon_hooks()
    import concourse.bass_utils as bass_utils
    bass_utils.upload_artifacts = lambda d: f"local:{d}"

    x = np.asarray(x, dtype=np.float32)
    W = np.asarray(W, dtype=np.float32)
    V = np.asarray(V, dtype=np.float32)
    b = np.asarray(b, dtype=np.float32)

    xT_bf = np.ascontiguousarray(x.T).astype(BF16)
    Vt_bf = V.transpose(1, 0, 2).astype(BF16)   # (U, D, E) contiguous bf16
    # Permute each unit's matrix to [p, k, e] (partition-major for the
    # K-chunked matmul layout) so the per-unit DMA is contiguous.
    Vt_bf = Vt_bf.reshape(U, KC, P, D).transpose(0, 2, 1, 3)
    W_bf = W.astype(BF16)

    in_maps = []
    for c in range(N_CORES):
        us, ue = c * UPC, (c + 1) * UPC
        in_maps.append({
            "xT": xT_bf,
            "x32": x,
            "Vs": np.ascontiguousarray(Vt_bf[us:ue]),
            "Ws": np.ascontiguousarray(W_bf[:, us:ue]),
            "bs": np.ascontiguousarray(
                np.broadcast_to(b[us:ue], (P, UPC))).astype(np.float32),
        })

    nc = _build_program()
    res = None
    last_exc = None
    for attempt in range(3):
        try:
            res = bass_utils.run_bass_kernel_spmd(
                nc, in_maps, core_ids=list(range(N_CORES)))
            break
        except Exception as e:  # transient NRT device errors have been seen
            last_exc = e
    if res is None:
        raise last_exc
    _LAST_RUN["result"] = res

    out = np.concatenate(
        [res.results[c]["outs"] for c in range(N_CORES)], axis=1)
    return out.astype(np.float32)
